# revision 1
# baseline (speedup 1.0000x reference)
"""Deformable Conv2d (B=8, C=64, H=W=128, O=64, K=3) on 8 trn2 NeuronCores.

Strategy (data-parallel over batch, one batch element per core):
  1. Offset conv on PE via 9 shifted-AP matmuls over a zero-padded x in SBUF.
  2. PE-transpose offsets to sample-major layout (x-coordinate on partitions).
  3. Phase-2 pointwise on DVE: sample coords, floors (via mod), bilinear
     corner weights with validity masks, int32 gather row indices.
  4. Streamed y-chunks: indirect DMA gathers 2 contiguous pixel-pairs per
     sample (512B descriptors, f32) from a pixel-major padded copy of x in
     DRAM; DVE multiplies the 4 corners by their weights; PE transposes the
     weighted corners with PSUM accumulation (the 4-corner bilinear sum
     happens in PSUM); main einsum as 5 accumulated matmuls; bias via ACT.

Everything is fp32 end-to-end; exact up to fp32 rounding.
"""

import numpy as np

import concourse.bass as bass
import concourse.bacc as bacc
import concourse.mybir as mybir
import concourse.tile as tile
from concourse import bass_utils

F32 = mybir.dt.float32
I32 = mybir.dt.int32
I16 = mybir.dt.int16

B, C, H, W, O, K = 8, 64, 128, 128, 64, 3
KK = K * K
P = H * W                      # 16384 pixels
PW = W + 2                     # 130: padded row width
PADC = 17184                   # x_pad columns (>= 16646 + 512)
OFFC = 17056                   # off_pad columns (>= 131 + 33*512)
NROW = P + 129 + 130           # x_pm rows (129 zero | 16384 data | 130 zero)
NS = KK * H                    # 1152 (k, y) groups; sample n = k*128 + y
YB = 8                         # y rows per transpose batch (stages B)
YC = 4                         # y rows per main-loop chunk
NCH = H // YC                  # 64 chunks
AluOp = mybir.AluOpType
ActF = mybir.ActivationFunctionType


def _consts():
    eye = np.eye(128, dtype=np.float32)
    ky, kx = np.meshgrid(np.arange(-1, 2), np.arange(-1, 2), indexing="ij")
    dky = ky.reshape(-1).astype(np.float32)
    dkx = kx.reshape(-1).astype(np.float32)
    yv = np.arange(H, dtype=np.float32)
    xv = np.arange(W, dtype=np.float32)
    # sample n = y*KK + k: by_c[x, n] = y + 1 + dky[k]; bx_c[x, n] = x + 1 + dkx[k]
    by = (yv[:, None] + 1.0 + dky[None, :]).reshape(1, NS)
    by_c = np.broadcast_to(by, (128, NS)).copy()
    bx = np.broadcast_to(dkx[None, :], (H, KK)).reshape(1, NS)
    bx_c = (xv[:, None] + bx + 1.0).astype(np.float32)
    return eye, by_c.astype(np.float32), bx_c


def _weights_layout(weight, off_w):
    # ow_l[c, s*18 + j] = off_w[j, c, ky, kx] with s = ky*3+kx
    ow = np.transpose(off_w.reshape(18, C, KK), (1, 2, 0))  # (C, s, j)
    ow_l = np.ascontiguousarray(ow.reshape(C, KK * 18)).astype(np.float32)
    # w2_l[c, k*64 + o] = weight[o, c, k]
    w2 = np.transpose(weight.reshape(O, C, KK), (1, 2, 0))  # (c, k, o)
    w2_l = np.ascontiguousarray(w2.reshape(C, KK * O)).astype(np.float32)
    return ow_l, w2_l


def build_module():
    nc = bacc.Bacc("TRN2", target_bir_lowering=False, debug=False,
                   enable_asserts=True)

    xin = nc.dram_tensor("xin", (C, P), F32, kind="ExternalInput").ap()
    ow_d = nc.dram_tensor("ow_l", (C, KK * 18), F32, kind="ExternalInput").ap()
    w2_d = nc.dram_tensor("w2_l", (C, KK * O), F32, kind="ExternalInput").ap()
    bias_d = nc.dram_tensor("bias", (O, 1), F32, kind="ExternalInput").ap()
    eye_d = nc.dram_tensor("eye", (128, 128), F32, kind="ExternalInput").ap()
    by_d = nc.dram_tensor("by_c", (128, NS), F32, kind="ExternalInput").ap()
    bx_d = nc.dram_tensor("bx_c", (128, NS), F32, kind="ExternalInput").ap()
    out_d = nc.dram_tensor("out", (O, P), F32, kind="ExternalOutput").ap()

    with tile.TileContext(nc) as tc:
        with (
            tc.tile_pool(name="const", bufs=1) as cpool,
            tc.tile_pool(name="ph2out", bufs=1) as wpool,
            tc.tile_pool(name="dram", bufs=1, space="DRAM") as dpool,
        ):
            x_pm_t = dpool.tile([NROW, C], F32)
            x_pm = x_pm_t[:]
            ident = cpool.tile([128, 128], F32)
            nc.sync.dma_start(out=ident[:], in_=eye_d)
            ow_sb = cpool.tile([C, KK * 18], F32)
            nc.sync.dma_start(out=ow_sb[:], in_=ow_d)
            w2_sb = cpool.tile([C, KK * O], F32)
            nc.sync.dma_start(out=w2_sb[:], in_=w2_d)
            bias_sb = cpool.tile([O, 1], F32)
            nc.sync.dma_start(out=bias_sb[:], in_=bias_d)

            w4 = wpool.tile([128, 4 * NS], F32)
            idx16 = wpool.tile([128, 16 * NS], I16)

            with tc.tile_pool(name="offt", bufs=1) as otpool:
                offt = otpool.tile([128, H * 18], F32)
                with tc.tile_pool(name="xpad", bufs=1) as xpool:
                    # ---- Stage A: padded x in SBUF ----
                    x_pad = xpool.tile([C, PW * PW + 160], F32)
                    nc.vector.memset(x_pad[:], 0.0)
                    xp3 = x_pad[:, 0 : PW * PW].rearrange("c (r w) -> c r w", w=PW)
                    nc.sync.dma_start(
                        out=xp3[:, 1 : H + 1, 1 : W + 1],
                        in_=xin.rearrange("c (h w) -> c h w", w=W),
                    )

                    # ---- Stage B: pixel-major padded x in DRAM ----
                    with (
                        tc.tile_pool(name="stgB", bufs=3) as spool,
                        tc.tile_pool(name="psB", bufs=2, space="PSUM") as psB,
                    ):
                        stg0 = spool.tile([128, YB * C], F32, tag="pmS")
                        nc.vector.memset(stg0[:], 0.0)
                        pm_flat = x_pm.rearrange("r c -> (r c)")
                        nc.sync.dma_start(
                            out=pm_flat[0 : 129 * C].rearrange(
                                "(a e) -> a e", a=32),
                            in_=stg0[0:32, 0:258])
                        nc.sync.dma_start(
                            out=pm_flat[(129 + P) * C :].rearrange(
                                "(a e) -> a e", a=32),
                            in_=stg0[0:32, 0:260])
                        for y0 in range(0, H, YB):
                            pst = psB.tile([128, YB * C], F32, tag="pmT")
                            for yy in range(YB):
                                y = y0 + yy
                                nc.tensor.matmul(
                                    out=pst[:, yy * C : (yy + 1) * C],
                                    lhsT=x_pad[:, (y + 1) * PW + 1 :
                                               (y + 1) * PW + 1 + W],
                                    rhs=ident[0:C, 0:C],
                                    is_transpose=True, start=True, stop=True,
                                )
                            stg = spool.tile([128, YB * C], F32, tag="pmS")
                            nc.scalar.copy(out=stg[:], in_=pst[:])
                            dst = x_pm[129 + y0 * W : 129 + (y0 + YB) * W, :]
                            dst = dst.rearrange("(yy x) c -> x yy c", x=W)
                            nc.sync.dma_start(
                                out=dst,
                                in_=stg[:].rearrange("x (yy c) -> x yy c", c=C))

                    # ---- Stage C+D: offset conv by row pairs -> OFFT ----
                    # off[j, p] for rows (2m, 2m+1); then transpose each row
                    shifts = [(ky - 1) * PW + (kx - 1)
                              for ky in range(K) for kx in range(K)]
                    with (
                        tc.tile_pool(name="rowb", bufs=3) as rpool,
                        tc.tile_pool(name="psC", bufs=2, space="PSUM") as psC,
                        tc.tile_pool(name="psD", bufs=2, space="PSUM") as psD,
                    ):
                        for m in range(H // 2):
                            cb = (2 * m + 1) * PW + 1  # padded idx of row 2m, x=0
                            pso = psC.tile([18, 2 * PW], F32, tag="ocP")
                            for s in range(KK):
                                nc.tensor.matmul(
                                    out=pso[:],
                                    lhsT=ow_sb[:, s * 18 : (s + 1) * 18],
                                    rhs=x_pad[:, cb + shifts[s] :
                                              cb + shifts[s] + 2 * PW],
                                    start=(s == 0), stop=(s == KK - 1),
                                )
                            rowb = rpool.tile([18, 2 * PW], F32, tag="rb")
                            nc.scalar.copy(out=rowb[:], in_=pso[:])
                            psd = psD.tile([128, 2 * 18], F32, tag="otP")
                            for yy in range(2):
                                nc.tensor.matmul(
                                    out=psd[:, yy * 18 : (yy + 1) * 18],
                                    lhsT=rowb[:, yy * PW : yy * PW + W],
                                    rhs=ident[0:18, 0:18],
                                    is_transpose=True, start=True, stop=True,
                                )
                            nc.scalar.copy(
                                out=offt[:, (2 * m) * 18 : (2 * m + 2) * 18],
                                in_=psd[:])

                # ---- Stage E: phase-2 pointwise (sample n = y*KK + k) ----
                with tc.tile_pool(name="ph2tmp", bufs=1) as ppool:
                    def oview(i):
                        a = offt[:]
                        return bass.AP(a.tensor, a.offset + i,
                                       [list(a.ap[0]), [18, H], [2, KK]])

                    by_sb = ppool.tile([128, NS], F32)
                    nc.sync.dma_start(out=by_sb[:], in_=by_d)
                    bx_sb = ppool.tile([128, NS], F32)
                    nc.sync.dma_start(out=bx_sb[:], in_=bx_d)

                    sy = ppool.tile([128, NS], F32)
                    sx = ppool.tile([128, NS], F32)
                    wy = ppool.tile([128, NS], F32)
                    wx = ppool.tile([128, NS], F32)
                    fy = ppool.tile([128, NS], F32)
                    fx = ppool.tile([128, NS], F32)
                    t0 = ppool.tile([128, NS], F32)
                    t1 = ppool.tile([128, NS], F32)
                    idxT = ppool.tile([128, 2 * NS], I16)
                    V = nc.vector
                    V.tensor_tensor(out=sy[:], in0=oview(0), in1=by_sb[:],
                                    op=AluOp.add)
                    V.tensor_tensor(out=sx[:], in0=oview(1), in1=bx_sb[:],
                                    op=AluOp.add)
                    # floor via fp32 magic rounding; integer ties harmless
                    MAGIC = 12582912.0  # 1.5 * 2^23
                    V.tensor_scalar(out=t0[:], in0=sy[:], scalar1=0.5,
                                    scalar2=None, op0=AluOp.subtract)
                    V.tensor_scalar(out=fy[:], in0=t0[:], scalar1=MAGIC,
                                    scalar2=MAGIC, op0=AluOp.add,
                                    op1=AluOp.subtract)
                    V.tensor_tensor(out=wy[:], in0=sy[:], in1=fy[:],
                                    op=AluOp.subtract)
                    V.tensor_scalar(out=t0[:], in0=sx[:], scalar1=0.5,
                                    scalar2=None, op0=AluOp.subtract)
                    V.tensor_scalar(out=fx[:], in0=t0[:], scalar1=MAGIC,
                                    scalar2=MAGIC, op0=AluOp.add,
                                    op1=AluOp.subtract)
                    V.tensor_tensor(out=wx[:], in0=sx[:], in1=fx[:],
                                    op=AluOp.subtract)
                    va, vb, vc, vd = sy, sx, by_sb, bx_sb

                    def masks(f, lo, out_t):
                        V.tensor_scalar(out=t0[:], in0=f[:], scalar1=float(lo),
                                        scalar2=None, op0=AluOp.is_ge)
                        V.tensor_scalar(out=t1[:], in0=f[:],
                                        scalar1=float(lo + 127),
                                        scalar2=None, op0=AluOp.is_le)
                        V.tensor_tensor(out=out_t[:], in0=t0[:], in1=t1[:],
                                        op=AluOp.mult)

                    masks(fy, 0, va)
                    V.tensor_scalar(out=t0[:], in0=wy[:], scalar1=-1.0,
                                    scalar2=1.0, op0=AluOp.mult, op1=AluOp.add)
                    V.tensor_tensor(out=va[:], in0=va[:], in1=t0[:],
                                    op=AluOp.mult)
                    masks(fy, -1, vb)
                    V.tensor_tensor(out=vb[:], in0=vb[:], in1=wy[:],
                                    op=AluOp.mult)
                    masks(fx, 0, vc)
                    V.tensor_scalar(out=t0[:], in0=wx[:], scalar1=-1.0,
                                    scalar2=1.0, op0=AluOp.mult, op1=AluOp.add)
                    V.tensor_tensor(out=vc[:], in0=vc[:], in1=t0[:],
                                    op=AluOp.mult)
                    masks(fx, -1, vd)
                    V.tensor_tensor(out=vd[:], in0=vd[:], in1=wx[:],
                                    op=AluOp.mult)

                    def w4view(i):
                        a = w4[:]
                        return bass.AP(a.tensor, a.offset + i,
                                       [list(a.ap[0]), [4, NS]])

                    V.tensor_tensor(out=w4view(0), in0=va[:], in1=vc[:],
                                    op=AluOp.mult)
                    V.tensor_tensor(out=w4view(1), in0=va[:], in1=vd[:],
                                    op=AluOp.mult)
                    V.tensor_tensor(out=w4view(2), in0=vb[:], in1=vc[:],
                                    op=AluOp.mult)
                    V.tensor_tensor(out=w4view(3), in0=vb[:], in1=vd[:],
                                    op=AluOp.mult)

                    # idx_top = clamp(fy,-1,127)*128 + clamp(fx,-1,127) + 129
                    idf = wy
                    V.tensor_scalar(out=t0[:], in0=fy[:], scalar1=-1.0,
                                    scalar2=127.0, op0=AluOp.max,
                                    op1=AluOp.min)
                    V.tensor_scalar(out=t1[:], in0=fx[:], scalar1=-1.0,
                                    scalar2=127.0, op0=AluOp.max,
                                    op1=AluOp.min)
                    V.tensor_scalar(out=t0[:], in0=t0[:], scalar1=128.0,
                                    scalar2=129.0, op0=AluOp.mult,
                                    op1=AluOp.add)
                    V.tensor_tensor(out=idf[:], in0=t0[:], in1=t1[:],
                                    op=AluOp.add)

                    def idxview(tb):
                        a = idxT[:]
                        return bass.AP(a.tensor, a.offset + tb,
                                       [list(a.ap[0]), [2, NS]])

                    V.tensor_copy(out=idxview(0), in_=idf[:])
                    V.tensor_scalar(out=idf[:], in0=idf[:], scalar1=128.0,
                                    scalar2=None, op0=AluOp.add)
                    V.tensor_copy(out=idxview(1), in_=idf[:])

                    # collapse idxT (128=x, n2) -> wrapped-16:
                    # idx16[x%16, n2*8 + x//16] = idxT[x, n2]
                    idxa = ppool.tile([16, 16 * NS], I16)
                    for a in range(8):
                        nc.sync.dma_start(
                            out=idxa[:, a * 2 * NS : (a + 1) * 2 * NS],
                            in_=idxT[16 * a : 16 * a + 16, :])
                    aa = idxa[:]
                    V.tensor_copy(
                        out=idx16[0:16, :],
                        in_=bass.AP(aa.tensor, aa.offset,
                                    [list(aa.ap[0]), [1, 2 * NS], [2 * NS, 8]]))
                    nc.sync.dma_start(out=idx16[16:32, :], in_=idx16[0:16, :])
                    nc.sync.dma_start(out=idx16[32:64, :], in_=idx16[0:32, :])
                    nc.sync.dma_start(out=idx16[64:128, :], in_=idx16[0:64, :])

            # ---- Stage F: streamed gather + lerp + transpose + einsum ----
            with (
                tc.tile_pool(name="gat", bufs=2) as gpool,
                tc.tile_pool(name="zt", bufs=2) as zpool,
                tc.tile_pool(name="ob", bufs=2) as obpool,
                tc.tile_pool(name="psZ", bufs=1, space="PSUM") as psZ,
                tc.tile_pool(name="psO", bufs=2, space="PSUM") as psO,
            ):
                V = nc.vector
                xa = x_pm.rearrange("r c -> (r c)")
                x_pairs = bass.AP(xa.tensor, xa.offset,
                                  [[C, NROW - 1], [1, 2 * C]])
                NIDX_CH = 128 * YC * KK * 2       # 9216 per chunk
                NCALLS = NIDX_CH // 1024          # 9 gather calls of 1024
                for ch in range(NCH):
                    y0 = ch * YC
                    G = gpool.tile([128, KK * YC * 2 * 128], F32, tag="G")
                    for q in range(NCALLS):
                        nc.gpsimd.dma_gather(
                            out_ap=G[:, q * 1024 * 128 // 128 :
                                     (q + 1) * 1024 * 128 // 128].rearrange(
                                "p (n e) -> p n e", e=128),
                            in_ap=x_pairs,
                            idxs_ap=idx16[:, y0 * KK * 16 + q * 64 :
                                          y0 * KK * 16 + (q + 1) * 64],
                            num_idxs=1024, num_idxs_reg=1024,
                            elem_size=2 * C, elem_step=C,
                        )
                    for hf in range(2):
                        hb = hf * 2 * KK * 2 * 128   # half base in G (elems)
                        # weighted corners, in place: G *= w4 (per corner)
                        ga, wa = G[:], w4[:]
                        for tb in range(2):
                            for j in range(2):
                                i = tb * 2 + j
                                gv = bass.AP(
                                    ga.tensor,
                                    ga.offset + hb + tb * 128 + j * 64,
                                    [list(ga.ap[0]), [KK * 2 * 128, 2],
                                     [2 * 128, KK], [1, C]])
                                wv = bass.AP(
                                    wa.tensor,
                                    wa.offset + 4 * ((y0 + 2 * hf) * KK) + i,
                                    [list(wa.ap[0]), [4 * KK, 2], [4, KK],
                                     [0, C]])
                                V.tensor_tensor(out=gv, in0=gv, in1=wv,
                                                op=AluOp.mult)
                        # transpose weighted corners, accumulate 4 into PSUM
                        pzA = psZ.tile([64, 4 * 2 * 128], F32, tag="pzA")
                        pzB = psZ.tile([64, 4 * 2 * 128], F32, tag="pzB")
                        pzC = psZ.tile([64, 2 * 128], F32, tag="pzC")
                        for k in range(KK):
                            pzt = (pzA, pzB, pzC)[k // 4]
                            cb0 = (k % 4) * 2 * 128 if k < 8 else 0
                            for yy in range(2):
                                for i in range(4):
                                    tb, j = i // 2, i % 2
                                    lo = (hb + ((yy * KK + k) * 2 + tb) * 128
                                          + j * 64)
                                    nc.tensor.matmul(
                                        out=pzt[:, cb0 + yy * 128 :
                                                cb0 + (yy + 1) * 128],
                                        lhsT=G[:, lo : lo + 64], rhs=ident[:],
                                        is_transpose=True,
                                        start=(i == 0), stop=(i == 3),
                                    )
                        Z = zpool.tile([64, KK * 2 * 128], F32, tag="Z")
                        nc.scalar.copy(out=Z[:, 0 : 4 * 2 * 128], in_=pzA[:])
                        nc.scalar.copy(out=Z[:, 4 * 2 * 128 : 8 * 2 * 128],
                                       in_=pzB[:])
                        nc.scalar.copy(out=Z[:, 8 * 2 * 128 :], in_=pzC[:])
                        # main einsum over (c, k)
                        po = psO.tile([O, 2 * 128], F32, tag="po")
                        for k in range(KK):
                            nc.tensor.matmul(
                                out=po[:], lhsT=w2_sb[:, k * O : (k + 1) * O],
                                rhs=Z[:, k * 2 * 128 : (k + 1) * 2 * 128],
                                start=(k == 0), stop=(k == KK - 1),
                            )
                        osb = obpool.tile([O, 2 * 128], F32, tag="osb")
                        nc.scalar.activation(
                            out=osb[:], in_=po[:], func=ActF.Identity,
                            bias=bias_sb[:], scale=1.0,
                        )
                        nc.sync.dma_start(
                            out=out_d[:, (y0 + 2 * hf) * W :
                                      (y0 + 2 * hf + 2) * W],
                            in_=osb[:])

    nc.compile()
    return nc


_NC_CACHE = {}


def _get_module():
    if "nc" not in _NC_CACHE:
        _NC_CACHE["nc"] = build_module()
    return _NC_CACHE["nc"]


def make_in_maps(x, weight, bias, off_w, off_b):
    x = np.ascontiguousarray(np.asarray(x, np.float32))
    weight = np.asarray(weight, np.float32)
    bias = np.asarray(bias, np.float32)
    off_w = np.asarray(off_w, np.float32)
    off_b = np.asarray(off_b, np.float32)

    eye, by_c, bx_c = _consts()
    ow_l, w2_l = _weights_layout(weight, off_w)
    # fold off_b into by/bx: off channel j = kk*2+i
    ob = off_b.reshape(KK, 2)
    by_c = (by_c + np.tile(ob[:, 0], H)[None, :]).astype(np.float32)
    bx_c = (bx_c + np.tile(ob[:, 1], H)[None, :]).astype(np.float32)

    shared = {
        "ow_l": ow_l, "w2_l": w2_l, "bias": bias.reshape(O, 1).copy(),
        "eye": eye, "by_c": by_c, "bx_c": bx_c,
    }
    in_maps = []
    for b in range(x.shape[0]):
        m = dict(shared)
        m["xin"] = np.ascontiguousarray(x[b].reshape(C, P))
        in_maps.append(m)
    return in_maps


def kernel(x, weight, bias, off_w, off_b):
    in_maps = make_in_maps(x, weight, bias, off_w, off_b)
    nc = _get_module()
    res = bass_utils.run_bass_kernel_spmd(nc, in_maps, core_ids=list(range(B)))
    out = np.stack([res.results[b]["out"].reshape(O, H, W) for b in range(B)])
    return out

